# revision 4
# baseline (speedup 1.0000x reference)
"""Soft-MoE discrete-action transition network — Trainium2 Bass kernel.

Problem shapes (hardcoded):
  obs [B=64, M=256, D=256] f32, action [B=64] i64,
  phi [D, E=4, S=64] f32, w1 [E, D, H=512] f32, b1 [E, H] f32 (zeros),
  w2 [E, H, A*D=4608] f32, b2 [E, A*D] f32 (zeros).  Output [B, M, D] f32.

Strategy:
  * Host gathers the action-selected slice of w2/b2 per batch element
    (w2sel[b] = w2[:, :, a_b*D:(a_b+1)*D]) — the one-hot contraction at the
    end of the reference selects exactly one D-wide block per batch, so
    doing the selection first cuts the dominant matmuls by A=18x.
  * Data-parallel over batch: 8 batch elements per NeuronCore, params
    replicated, no collectives.
  * Per batch, on device (P=128 partition chunks):
      logits  [m,es] = obsT.T @ phi      (lhsT=obsT[d,m], rhs=phi[d,es])
      logitsT [es,m] = phi.T  @ obsT     (lhsT=phi, rhs=obsT — same operands)
      exp both (ScalarE, accum_out gives the softmax denominators for free)
      slotsT  [d,es] = obs.T @ exp_l     (unnormalized dispatch)
      pre_h   [h,es] = w1_e.T @ slotsT   per expert; ReLU (dispatch softmax
                        normalizer folded past ReLU — valid since b1 == 0)
      y       [es,d] = h_e.T @ w2sel_e; scale rows by 1/colsum (dispatch)
      out     [m,d]  = exp_lT.T @ y; scale rows by 1/rowsum (combine)
"""

import os
import sys
import time

import numpy as np

for _p in ("/opt/trn_rl_repo",):
    if os.path.isdir(_p) and _p not in sys.path:
        sys.path.append(_p)

import concourse.bass as bass
import concourse.mybir as mybir
import concourse.tile as tile
from concourse import bacc
from concourse.bass import ds, ts

B, M, D, A = 64, 256, 256, 18
E, S, H = 4, 64, 512
ES = E * S
N_CORES = 8
BPC = B // N_CORES  # batches per core
P = 128
F32 = mybir.dt.float32

AF = mybir.ActivationFunctionType


def build_nc(mm_dt=F32, has_b2=False):
    """Build the per-core Bass program (one NeuronCore, BPC batches)."""
    nc = bacc.Bacc("TRN2", target_bir_lowering=False, debug=False)

    # All tensors are pre-rearranged on the host into the exact SBUF layouts,
    # so every DMA is a contiguous [128, N] copy.
    obs_d = nc.dram_tensor("obs", [BPC, P, 2 * D], F32, kind="ExternalInput").ap()
    obsT_d = nc.dram_tensor("obsT", [BPC, P, 2 * M], F32, kind="ExternalInput").ap()
    phi_d = nc.dram_tensor("phi", [P, 2 * ES], F32, kind="ExternalInput").ap()
    w1_d = nc.dram_tensor("w1", [P, 2 * E * H], F32, kind="ExternalInput").ap()
    w2_d = nc.dram_tensor(
        "w2sel", [BPC, P, E * 4 * D], F32, kind="ExternalInput"
    ).ap()
    if has_b2:
        b2_d = nc.dram_tensor("b2sel", [BPC, E, D], F32, kind="ExternalInput").ap()
    out_d = nc.dram_tensor("out", [BPC, P, 2 * D], F32, kind="ExternalOutput").ap()

    def mm(ap):
        return ap if mm_dt == F32 else ap.bitcast(mm_dt)

    with tile.TileContext(nc) as tc:
        with (
            tc.tile_pool(name="const", bufs=1) as const,
            tc.tile_pool(name="io", bufs=2) as io,
            tc.tile_pool(name="mid", bufs=2) as mid,
            tc.tile_pool(name="psum", bufs=1, space="PSUM") as psp,
        ):
            phi_sb = const.tile([P, 2, ES], F32)
            nc.sync.dma_start(out=phi_sb, in_=phi_d)
            w1_sb = const.tile([P, 2, E, H], F32)
            nc.sync.dma_start(out=w1_sb, in_=w1_d)
            if has_b2:
                ones_sb = const.tile([1, S], F32)
                nc.vector.memset(ones_sb, 1.0)

            for ib in range(BPC):
                obs_sb = io.tile([P, 2, D], F32, tag="obs")
                nc.sync.dma_start(out=obs_sb, in_=obs_d[ib])
                obsT_sb = io.tile([P, 2, M], F32, tag="obsT")
                nc.sync.dma_start(out=obsT_sb, in_=obsT_d[ib])
                w2_sb = io.tile([P, E, 4, D], F32, tag="w2")
                nc.sync.dma_start(out=w2_sb, in_=w2_d[ib])
                if has_b2:
                    b2_sb = io.tile([E, D], F32, tag="b2")
                    nc.sync.dma_start(out=b2_sb, in_=b2_d[ib])

                # logits [m, es] (2 m-chunks), contracting d (2 chunks)
                lg_ps = psp.tile([P, 2, ES], F32, tag="lg")
                for mc in range(2):
                    for dc in range(2):
                        nc.tensor.matmul(
                            lg_ps[:, mc, :],
                            mm(obsT_sb[:, dc, ts(mc, P)]),
                            mm(phi_sb[:, dc, :]),
                            start=(dc == 0),
                            stop=(dc == 1),
                        )
                exp_l = mid.tile([P, 2, ES], F32, tag="expl")
                rsum = mid.tile([P, 2], F32, tag="rsum")
                for mc in range(2):
                    nc.scalar.activation(
                        exp_l[:, mc, :], lg_ps[:, mc, :], AF.Exp,
                        accum_out=rsum[:, mc : mc + 1],
                    )

                # logitsT [es, m] (2 es-chunks)
                lgT_ps = psp.tile([P, 2, M], F32, tag="lgT")
                for ec in range(2):
                    for dc in range(2):
                        nc.tensor.matmul(
                            lgT_ps[:, ec, :],
                            mm(phi_sb[:, dc, ts(ec, P)]),
                            mm(obsT_sb[:, dc, :]),
                            start=(dc == 0),
                            stop=(dc == 1),
                        )
                exp_lT = mid.tile([P, 2, M], F32, tag="explT")
                csum = mid.tile([P, 2], F32, tag="csum")
                for ec in range(2):
                    nc.scalar.activation(
                        exp_lT[:, ec, :], lgT_ps[:, ec, :], AF.Exp,
                        accum_out=csum[:, ec : ec + 1],
                    )

                recip_c = mid.tile([P, 2], F32, tag="rc")
                nc.vector.reciprocal(recip_c, rsum)
                recip_d = mid.tile([P, 2], F32, tag="rd")
                nc.vector.reciprocal(recip_d, csum)

                # slotsT [d, es] = obs.T @ exp_l (unnormalized dispatch)
                sl_ps = psp.tile([P, 2, ES], F32, tag="sl")
                for dc in range(2):
                    for mc in range(2):
                        nc.tensor.matmul(
                            sl_ps[:, dc, :],
                            mm(obs_sb[:, mc, ts(dc, P)]),
                            mm(exp_l[:, mc, :]),
                            start=(mc == 0),
                            stop=(mc == 1),
                        )
                slots_sb = mid.tile([P, 2, ES], F32, tag="slots")
                nc.vector.tensor_copy(slots_sb, sl_ps)

                # pre_h [h, (e,s)] per h-chunk; 4 h-chunks x 4 experts x 2 dc
                ph_ps = psp.tile([P, 4, ES], F32, tag="ph")
                for hc in range(4):
                    for e in range(E):
                        for dc in range(2):
                            nc.tensor.matmul(
                                ph_ps[:, hc, ds(e * S, S)],
                                mm(w1_sb[:, dc, e, ts(hc, P)]),
                                mm(slots_sb[:, dc, ds(e * S, S)]),
                                start=(dc == 0),
                                stop=(dc == 1),
                            )
                h_sb = mid.tile([P, 4, ES], F32, tag="h")
                nc.vector.tensor_scalar_max(h_sb, ph_ps, 0.0)

                # y [es, d]: expert e -> es-chunk e//2, partition off (e%2)*64
                y_ps = psp.tile([P, 2, D], F32, tag="y")
                for e in range(E):
                    ec, po = e // 2, (e % 2) * S
                    if has_b2:
                        nc.tensor.matmul(
                            y_ps[po : po + S, ec, :],
                            mm(ones_sb[:, :]),
                            mm(b2_sb[e : e + 1, :]),
                            start=True,
                            stop=False,
                        )
                    for hc in range(4):
                        nc.tensor.matmul(
                            y_ps[po : po + S, ec, :],
                            mm(h_sb[:, hc, ds(e * S, S)]),
                            mm(w2_sb[:, e, hc, :]),
                            start=(hc == 0 and not has_b2),
                            stop=(hc == 3),
                        )
                y_sb = mid.tile([P, 2, D], F32, tag="ysb")
                for ec in range(2):
                    nc.vector.tensor_scalar_mul(
                        y_sb[:, ec, :], in0=y_ps[:, ec, :],
                        scalar1=recip_d[:, ec : ec + 1],
                    )

                # out [m, d] = exp_lT.T @ y, then combine normalization
                ou_ps = psp.tile([P, 2, D], F32, tag="ou")
                for mc in range(2):
                    for ec in range(2):
                        nc.tensor.matmul(
                            ou_ps[:, mc, :],
                            mm(exp_lT[:, ec, ts(mc, P)]),
                            mm(y_sb[:, ec, :]),
                            start=(ec == 0),
                            stop=(ec == 1),
                        )
                out_sb = io.tile([P, 2, D], F32, tag="out")
                for mc in range(2):
                    nc.vector.tensor_scalar_mul(
                        out_sb[:, mc, :], in0=ou_ps[:, mc, :],
                        scalar1=recip_c[:, mc : mc + 1],
                    )
                nc.sync.dma_start(out=out_d[ib], in_=out_sb)

    nc.compile()
    return nc


class _Runner:
    """Compile once per process; re-execute via a cached jitted shard_map."""

    def __init__(self, mm_dt=F32, has_b2=False):
        self.nc = build_nc(mm_dt=mm_dt, has_b2=has_b2)
        self.has_b2 = has_b2
        self._fn = None

    def _build_fn(self):
        import jax
        from jax.sharding import Mesh, PartitionSpec
        from jax.experimental.shard_map import shard_map
        from concourse import bass2jax
        from concourse.bass2jax import _bass_exec_p, partition_id_tensor

        bass2jax.install_neuronx_cc_hook()
        nc = self.nc
        partition_name = (
            nc.partition_id_tensor.name if nc.partition_id_tensor else None
        )
        in_names, out_names, out_avals, zero_outs = [], [], [], []
        for alloc in nc.m.functions[0].allocations:
            if not isinstance(alloc, mybir.MemoryLocationSet):
                continue
            name = alloc.memorylocations[0].name
            if alloc.kind == "ExternalInput":
                if name != partition_name:
                    in_names.append(name)
            elif alloc.kind == "ExternalOutput":
                shape = tuple(alloc.tensor_shape)
                dtype = mybir.dt.np(alloc.dtype)
                out_names.append(name)
                out_avals.append(jax.core.ShapedArray(shape, dtype))
                zero_outs.append(np.zeros(shape, dtype))
        n_params = len(in_names)
        all_in_names = list(in_names) + list(out_names)
        if partition_name is not None:
            all_in_names.append(partition_name)

        def _body(*args):
            operands = list(args)
            if partition_name is not None:
                operands.append(partition_id_tensor())
            outs = _bass_exec_p.bind(
                *operands,
                out_avals=tuple(out_avals),
                in_names=tuple(all_in_names),
                out_names=tuple(out_names),
                lowering_input_output_aliases=(),
                sim_require_finite=True,
                sim_require_nnan=True,
                nc=nc,
            )
            return tuple(outs)

        devices = jax.devices()[:N_CORES]
        mesh = Mesh(np.asarray(devices), ("core",))
        n_outs = len(out_names)
        sharded = jax.jit(
            shard_map(
                _body,
                mesh=mesh,
                in_specs=(PartitionSpec("core"),) * (n_params + n_outs),
                out_specs=(PartitionSpec("core"),) * n_outs,
                check_rep=False,
            ),
            donate_argnums=tuple(range(n_params, n_params + n_outs)),
            keep_unused=True,
        )
        self._in_names = in_names
        self._out_names = out_names
        self._out_avals = out_avals
        self._zero_outs = zero_outs
        self._fn = sharded

    def run(self, in_maps):
        """in_maps: list of N_CORES dicts name->np.ndarray. Returns per-core
        dict of outputs."""
        if self._fn is None:
            self._build_fn()
        concat_in = [
            np.concatenate([in_maps[c][nm] for c in range(N_CORES)], axis=0)
            for nm in self._in_names
        ]
        concat_zeros = [
            np.zeros((N_CORES * z.shape[0], *z.shape[1:]), z.dtype)
            for z in self._zero_outs
        ]
        out_arrs = self._fn(*concat_in, *concat_zeros)
        return [
            {
                nm: np.asarray(out_arrs[i]).reshape(
                    N_CORES, *self._out_avals[i].shape
                )[c]
                for i, nm in enumerate(self._out_names)
            }
            for c in range(N_CORES)
        ]


_runner_cache = {}


def _prep_inputs(obs, action, phi, w1, b1, w2, b2):
    obs = np.ascontiguousarray(np.asarray(obs, dtype=np.float32))
    action = np.asarray(action).astype(np.int64)
    phi = np.asarray(phi, dtype=np.float32)
    w1 = np.ascontiguousarray(np.asarray(w1, dtype=np.float32))
    b1 = np.asarray(b1, dtype=np.float32)
    w2 = np.asarray(w2, dtype=np.float32)
    b2 = np.asarray(b2, dtype=np.float32)
    if np.any(b1):
        raise NotImplementedError(
            "kernel assumes b1 == 0 (dispatch normalizer is folded past ReLU)"
        )
    obsT = obs.transpose(0, 2, 1)
    # Pre-rearrange everything into the kernel's SBUF layouts (partition dim
    # first, contiguous free) so on-device DMAs are plain [128, N] copies.
    # obs [B,M,D] -> (b, p, mc, d): m = mc*128 + p
    obs_k = np.ascontiguousarray(
        obs.reshape(B, 2, P, D).transpose(0, 2, 1, 3)
    ).reshape(B, P, 2 * D)
    # obsT [B,D,M] -> (b, p, dc, m): d = dc*128 + p
    obsT_k = np.ascontiguousarray(
        obsT.reshape(B, 2, P, M).transpose(0, 2, 1, 3)
    ).reshape(B, P, 2 * M)
    # phi [D,ES] -> (p, dc, es)
    phi_k = np.ascontiguousarray(
        phi.reshape(2, P, ES).transpose(1, 0, 2)
    ).reshape(P, 2 * ES)
    # w1 [E,D,H] -> (p, dc, e, h)
    w1_k = np.ascontiguousarray(
        w1.reshape(E, 2, P, H).transpose(2, 1, 0, 3)
    ).reshape(P, 2 * E * H)
    # per-batch action-selected slices: w2sel [B,E,H,D] -> (b, p, e, hc, d)
    w2r = w2.reshape(E, H, A, D)
    w2sel = w2r[:, :, action, :].transpose(2, 0, 1, 3)  # [B,E,H,D]
    w2_k = np.ascontiguousarray(
        w2sel.reshape(B, E, 4, P, D).transpose(0, 3, 1, 2, 4)
    ).reshape(B, P, E * 4 * D)
    has_b2 = bool(np.any(b2))
    b2_k = None
    if has_b2:
        b2r = b2.reshape(E, A, D)
        b2_k = np.ascontiguousarray(b2r[:, action, :].transpose(1, 0, 2))

    in_maps = []
    for c in range(N_CORES):
        sl = slice(c * BPC, (c + 1) * BPC)
        m = {
            "obs": obs_k[sl],
            "obsT": obsT_k[sl],
            "phi": phi_k,
            "w1": w1_k,
            "w2sel": w2_k[sl],
        }
        if has_b2:
            m["b2sel"] = b2_k[sl]
        in_maps.append(m)
    return in_maps, has_b2


def get_runner(has_b2, mm_dt=F32):
    key = (str(mm_dt), has_b2)
    if key not in _runner_cache:
        _runner_cache[key] = _Runner(mm_dt=mm_dt, has_b2=has_b2)
    return _runner_cache[key]


def kernel(obs, action, phi, w1, b1, w2, b2):
    in_maps, has_b2 = _prep_inputs(obs, action, phi, w1, b1, w2, b2)
    runner = get_runner(has_b2)
    results = runner.run(in_maps)
    out_k = np.concatenate([results[c]["out"] for c in range(N_CORES)], axis=0)
    # (b, p, mc, d) -> [B, M, D] with m = mc*128 + p
    out = out_k.reshape(B, P, 2, D).transpose(0, 2, 1, 3).reshape(B, M, D)
    return np.ascontiguousarray(out).astype(np.float32)


# revision 9
# speedup vs baseline: 1.1598x; 1.1598x over previous
"""Soft-MoE discrete-action transition network — Trainium2 Bass kernel.

Problem shapes (hardcoded):
  obs [B=64, M=256, D=256] f32, action [B=64] i64,
  phi [D, E=4, S=64] f32, w1 [E, D, H=512] f32, b1 [E, H] f32 (zeros),
  w2 [E, H, A*D=4608] f32, b2 [E, A*D] f32 (zeros).  Output [B, M, D] f32.

Strategy:
  * Host gathers the action-selected slice of w2/b2 per batch element
    (w2sel[b] = w2[:, :, a_b*D:(a_b+1)*D]) — the one-hot contraction at the
    end of the reference selects exactly one D-wide block per batch, so
    doing the selection first cuts the dominant matmuls by A=18x.
  * Data-parallel over batch: 8 batch elements per NeuronCore, params
    replicated, no collectives.
  * Per batch, on device (P=128 partition chunks):
      logits  [m,es] = obsT.T @ phi      (lhsT=obsT[d,m], rhs=phi[d,es])
      logitsT [es,m] = phi.T  @ obsT     (lhsT=phi, rhs=obsT — same operands)
      exp both (ScalarE, accum_out gives the softmax denominators for free)
      slotsT  [d,es] = obs.T @ exp_l     (unnormalized dispatch)
      pre_h   [h,es] = w1_e.T @ slotsT   per expert; ReLU (dispatch softmax
                        normalizer folded past ReLU — valid since b1 == 0)
      y       [es,d] = h_e.T @ w2sel_e; scale rows by 1/colsum (dispatch)
      out     [m,d]  = exp_lT.T @ y; scale rows by 1/rowsum (combine)
"""

import os
import sys
import time

import numpy as np

for _p in ("/opt/trn_rl_repo",):
    if os.path.isdir(_p) and _p not in sys.path:
        sys.path.append(_p)

import concourse.bass as bass
import concourse.mybir as mybir
import concourse.tile as tile
from concourse import bacc
from concourse.bass import ds, ts

B, M, D, A = 64, 256, 256, 18
E, S, H = 4, 64, 512
ES = E * S
N_CORES = 8
BPC = B // N_CORES  # batches per core
P = 128
F32 = mybir.dt.float32

AF = mybir.ActivationFunctionType

# Matmul operand dtype: float32r reinterprets fp32 operands for the PE's
# fast path (1 cycle/row at n>=256 vs 4 for plain fp32).
_MM_DT_ENV = os.environ.get("MOE_MM_DT", "float32r")
MM_DT = getattr(mybir.dt, _MM_DT_ENV)


def build_nc(mm_dt=F32, has_b2=False):
    """Build the per-core Bass program (one NeuronCore, BPC batches)."""
    nc = bacc.Bacc("TRN2", target_bir_lowering=False, debug=False)

    # All tensors are pre-rearranged on the host into the exact SBUF layouts,
    # so every DMA is a contiguous [128, N] copy.
    obs_d = nc.dram_tensor("obs", [BPC, P, 2 * D], mm_dt, kind="ExternalInput").ap()
    obsT_d = nc.dram_tensor(
        "obsT", [BPC, P, 2 * M], mm_dt, kind="ExternalInput"
    ).ap()
    phi_d = nc.dram_tensor("phi", [P, 2 * ES], mm_dt, kind="ExternalInput").ap()
    w1_d = nc.dram_tensor("w1", [P, 2 * E * H], mm_dt, kind="ExternalInput").ap()
    w2_d = nc.dram_tensor(
        "w2sel", [BPC, P, E * 4 * D], mm_dt, kind="ExternalInput"
    ).ap()
    if has_b2:
        b2_d = nc.dram_tensor(
            "b2sel", [BPC, E, D], mm_dt, kind="ExternalInput"
        ).ap()
    out_d = nc.dram_tensor("out", [BPC, P, 2 * D], F32, kind="ExternalOutput").ap()

    with tile.TileContext(nc) as tc:
        with (
            tc.tile_pool(name="const", bufs=1) as const,
            tc.tile_pool(name="io", bufs=2) as io,
            tc.tile_pool(name="mid", bufs=2) as mid,
            tc.tile_pool(name="psum", bufs=1, space="PSUM") as psp,
        ):
            phi_sb = const.tile([P, 2, ES], mm_dt)
            nc.sync.dma_start(out=phi_sb, in_=phi_d)
            w1_sb = const.tile([P, 2, E, H], mm_dt)
            nc.sync.dma_start(out=w1_sb, in_=w1_d)
            if has_b2:
                ones_sb = const.tile([1, S], mm_dt)
                nc.vector.memset(ones_sb, 1.0)

            for ib in range(BPC):
                obs_sb = io.tile([P, 2, D], mm_dt, tag="obs")
                nc.sync.dma_start(out=obs_sb, in_=obs_d[ib])
                obsT_sb = io.tile([P, 2, M], mm_dt, tag="obsT")
                nc.sync.dma_start(out=obsT_sb, in_=obsT_d[ib])
                w2_sb = io.tile([P, E, 4, D], mm_dt, tag="w2")
                nc.sync.dma_start(out=w2_sb, in_=w2_d[ib])
                if has_b2:
                    b2_sb = io.tile([E, D], mm_dt, tag="b2")
                    nc.sync.dma_start(out=b2_sb, in_=b2_d[ib])

                # logits [m, es] (2 m-chunks), contracting d (2 chunks)
                lg_ps = psp.tile([P, 2, ES], F32, tag="lg")
                for mc in range(2):
                    for dc in range(2):
                        nc.tensor.matmul(
                            lg_ps[:, mc, :],
                            obsT_sb[:, dc, ts(mc, P)],
                            phi_sb[:, dc, :],
                            start=(dc == 0),
                            stop=(dc == 1),
                        )
                exp_l = mid.tile([P, 2, ES], mm_dt, tag="expl")
                rsum = mid.tile([P, 2], F32, tag="rsum")
                for mc in range(2):
                    nc.scalar.activation(
                        exp_l[:, mc, :], lg_ps[:, mc, :], AF.Exp,
                        accum_out=rsum[:, mc : mc + 1],
                    )

                # logitsT [es, m] (2 es-chunks)
                lgT_ps = psp.tile([P, 2, M], F32, tag="lgT")
                for ec in range(2):
                    for dc in range(2):
                        nc.tensor.matmul(
                            lgT_ps[:, ec, :],
                            phi_sb[:, dc, ts(ec, P)],
                            obsT_sb[:, dc, :],
                            start=(dc == 0),
                            stop=(dc == 1),
                        )
                exp_lT = mid.tile([P, 2, M], mm_dt, tag="explT")
                csum = mid.tile([P, 2], F32, tag="csum")
                for ec in range(2):
                    nc.scalar.activation(
                        exp_lT[:, ec, :], lgT_ps[:, ec, :], AF.Exp,
                        accum_out=csum[:, ec : ec + 1],
                    )

                recip_c = mid.tile([P, 2], F32, tag="rc")
                nc.vector.reciprocal(recip_c, rsum)
                recip_d = mid.tile([P, 2], F32, tag="rd")
                nc.vector.reciprocal(recip_d, csum)

                # slotsT [d, es] = obs.T @ exp_l (unnormalized dispatch)
                sl_ps = psp.tile([P, 2, ES], F32, tag="sl")
                for dc in range(2):
                    for mc in range(2):
                        nc.tensor.matmul(
                            sl_ps[:, dc, :],
                            obs_sb[:, mc, ts(dc, P)],
                            exp_l[:, mc, :],
                            start=(mc == 0),
                            stop=(mc == 1),
                        )
                slots_sb = mid.tile([P, 2, ES], mm_dt, tag="slots")
                nc.vector.tensor_copy(slots_sb, sl_ps)

                # pre_h [h, (e,s)] per h-chunk; 4 h-chunks x 4 experts x 2 dc
                ph_ps = psp.tile([P, 4, ES], F32, tag="ph")
                for hc in range(4):
                    for e in range(E):
                        for dc in range(2):
                            nc.tensor.matmul(
                                ph_ps[:, hc, ds(e * S, S)],
                                w1_sb[:, dc, e, ts(hc, P)],
                                slots_sb[:, dc, ds(e * S, S)],
                                start=(dc == 0),
                                stop=(dc == 1),
                            )
                h_sb = mid.tile([P, 4, ES], mm_dt, tag="h")
                nc.vector.tensor_scalar_max(h_sb, ph_ps, 0.0)

                # y [es, d]: expert e -> es-chunk e//2, partition off (e%2)*64
                # (f32r matmuls must write PSUM partition base 0, so each
                # expert gets its own [64, D] PSUM tile, then VectorE applies
                # the dispatch normalizer while placing it into y_sb rows.)
                y_sb = mid.tile([P, 2, D], mm_dt, tag="ysb")
                for e in range(E):
                    ec, po = e // 2, (e % 2) * S
                    y_ps = psp.tile([S, D], F32, tag="y", bufs=2)
                    if has_b2:
                        nc.tensor.matmul(
                            y_ps,
                            ones_sb[:, :],
                            b2_sb[e : e + 1, :],
                            start=True,
                            stop=False,
                        )
                    for hc in range(4):
                        nc.tensor.matmul(
                            y_ps,
                            h_sb[:, hc, ds(e * S, S)],
                            w2_sb[:, e, hc, :],
                            start=(hc == 0 and not has_b2),
                            stop=(hc == 3),
                        )
                    nc.vector.tensor_scalar_mul(
                        y_sb[po : po + S, ec, :], in0=y_ps,
                        scalar1=recip_d[po : po + S, ec : ec + 1],
                    )

                # out [m, d] = exp_lT.T @ y, then combine normalization
                ou_ps = psp.tile([P, 2, D], F32, tag="ou")
                for mc in range(2):
                    for ec in range(2):
                        nc.tensor.matmul(
                            ou_ps[:, mc, :],
                            exp_lT[:, ec, ts(mc, P)],
                            y_sb[:, ec, :],
                            start=(ec == 0),
                            stop=(ec == 1),
                        )
                out_sb = io.tile([P, 2, D], F32, tag="out")
                for mc in range(2):
                    nc.vector.tensor_scalar_mul(
                        out_sb[:, mc, :], in0=ou_ps[:, mc, :],
                        scalar1=recip_c[:, mc : mc + 1],
                    )
                nc.sync.dma_start(out=out_d[ib], in_=out_sb)

    nc.compile()
    return nc


class _Runner:
    """Compile once per process; re-execute via a cached jitted shard_map."""

    def __init__(self, mm_dt=F32, has_b2=False):
        self.nc = build_nc(mm_dt=mm_dt, has_b2=has_b2)
        self.has_b2 = has_b2
        self._fn = None

    def _build_fn(self):
        import jax
        from jax.sharding import Mesh, PartitionSpec
        from jax.experimental.shard_map import shard_map
        from concourse import bass2jax
        from concourse.bass2jax import _bass_exec_p, partition_id_tensor

        bass2jax.install_neuronx_cc_hook()
        nc = self.nc
        partition_name = (
            nc.partition_id_tensor.name if nc.partition_id_tensor else None
        )
        in_names, out_names, out_avals, zero_outs = [], [], [], []
        for alloc in nc.m.functions[0].allocations:
            if not isinstance(alloc, mybir.MemoryLocationSet):
                continue
            name = alloc.memorylocations[0].name
            if alloc.kind == "ExternalInput":
                if name != partition_name:
                    in_names.append(name)
            elif alloc.kind == "ExternalOutput":
                shape = tuple(alloc.tensor_shape)
                dtype = mybir.dt.np(alloc.dtype)
                out_names.append(name)
                out_avals.append(jax.core.ShapedArray(shape, dtype))
                zero_outs.append(np.zeros(shape, dtype))
        n_params = len(in_names)
        all_in_names = list(in_names) + list(out_names)
        if partition_name is not None:
            all_in_names.append(partition_name)

        def _body(*args):
            operands = list(args)
            if partition_name is not None:
                operands.append(partition_id_tensor())
            outs = _bass_exec_p.bind(
                *operands,
                out_avals=tuple(out_avals),
                in_names=tuple(all_in_names),
                out_names=tuple(out_names),
                lowering_input_output_aliases=(),
                sim_require_finite=True,
                sim_require_nnan=True,
                nc=nc,
            )
            return tuple(outs)

        devices = jax.devices()[:N_CORES]
        mesh = Mesh(np.asarray(devices), ("core",))
        n_outs = len(out_names)
        sharded = jax.jit(
            shard_map(
                _body,
                mesh=mesh,
                in_specs=(PartitionSpec("core"),) * (n_params + n_outs),
                out_specs=(PartitionSpec("core"),) * n_outs,
                check_rep=False,
            ),
            donate_argnums=tuple(range(n_params, n_params + n_outs)),
            keep_unused=True,
        )
        self._in_names = in_names
        self._out_names = out_names
        self._out_avals = out_avals
        self._zero_outs = zero_outs
        self._fn = sharded

    def run(self, in_maps):
        """in_maps: list of N_CORES dicts name->np.ndarray. Returns per-core
        dict of outputs."""
        if self._fn is None:
            self._build_fn()
        concat_in = [
            np.concatenate([in_maps[c][nm] for c in range(N_CORES)], axis=0)
            for nm in self._in_names
        ]
        concat_zeros = [
            np.zeros((N_CORES * z.shape[0], *z.shape[1:]), z.dtype)
            for z in self._zero_outs
        ]
        out_arrs = self._fn(*concat_in, *concat_zeros)
        return [
            {
                nm: np.asarray(out_arrs[i]).reshape(
                    N_CORES, *self._out_avals[i].shape
                )[c]
                for i, nm in enumerate(self._out_names)
            }
            for c in range(N_CORES)
        ]


_runner_cache = {}


def _prep_inputs(obs, action, phi, w1, b1, w2, b2):
    obs = np.ascontiguousarray(np.asarray(obs, dtype=np.float32))
    action = np.asarray(action).astype(np.int64)
    phi = np.asarray(phi, dtype=np.float32)
    w1 = np.ascontiguousarray(np.asarray(w1, dtype=np.float32))
    b1 = np.asarray(b1, dtype=np.float32)
    w2 = np.asarray(w2, dtype=np.float32)
    b2 = np.asarray(b2, dtype=np.float32)
    if np.any(b1):
        raise NotImplementedError(
            "kernel assumes b1 == 0 (dispatch normalizer is folded past ReLU)"
        )
    obsT = obs.transpose(0, 2, 1)
    # Pre-rearrange everything into the kernel's SBUF layouts (partition dim
    # first, contiguous free) so on-device DMAs are plain [128, N] copies.
    # obs [B,M,D] -> (b, p, mc, d): m = mc*128 + p
    obs_k = np.ascontiguousarray(
        obs.reshape(B, 2, P, D).transpose(0, 2, 1, 3)
    ).reshape(B, P, 2 * D)
    # obsT [B,D,M] -> (b, p, dc, m): d = dc*128 + p
    obsT_k = np.ascontiguousarray(
        obsT.reshape(B, 2, P, M).transpose(0, 2, 1, 3)
    ).reshape(B, P, 2 * M)
    # phi [D,ES] -> (p, dc, es)
    phi_k = np.ascontiguousarray(
        phi.reshape(2, P, ES).transpose(1, 0, 2)
    ).reshape(P, 2 * ES)
    # w1 [E,D,H] -> (p, dc, e, h)
    w1_k = np.ascontiguousarray(
        w1.reshape(E, 2, P, H).transpose(2, 1, 0, 3)
    ).reshape(P, 2 * E * H)
    # per-batch action-selected slices: w2sel [B,E,H,D] -> (b, p, e, hc, d)
    w2r = w2.reshape(E, H, A, D)
    w2sel = w2r[:, :, action, :].transpose(2, 0, 1, 3)  # [B,E,H,D]
    w2_k = np.ascontiguousarray(
        w2sel.reshape(B, E, 4, P, D).transpose(0, 3, 1, 2, 4)
    ).reshape(B, P, E * 4 * D)
    has_b2 = bool(np.any(b2))
    b2_k = None
    if has_b2:
        b2r = b2.reshape(E, A, D)
        b2_k = np.ascontiguousarray(b2r[:, action, :].transpose(1, 0, 2))

    in_maps = []
    for c in range(N_CORES):
        sl = slice(c * BPC, (c + 1) * BPC)
        m = {
            "obs": obs_k[sl],
            "obsT": obsT_k[sl],
            "phi": phi_k,
            "w1": w1_k,
            "w2sel": w2_k[sl],
        }
        if has_b2:
            m["b2sel"] = b2_k[sl]
        in_maps.append(m)
    return in_maps, has_b2


def get_runner(has_b2, mm_dt=None):
    if mm_dt is None:
        mm_dt = MM_DT
    key = (str(mm_dt), has_b2)
    if key not in _runner_cache:
        _runner_cache[key] = _Runner(mm_dt=mm_dt, has_b2=has_b2)
    return _runner_cache[key]


def kernel(obs, action, phi, w1, b1, w2, b2):
    in_maps, has_b2 = _prep_inputs(obs, action, phi, w1, b1, w2, b2)
    runner = get_runner(has_b2)
    results = runner.run(in_maps)
    out_k = np.concatenate([results[c]["out"] for c in range(N_CORES)], axis=0)
    # (b, p, mc, d) -> [B, M, D] with m = mc*128 + p
    out = out_k.reshape(B, P, 2, D).transpose(0, 2, 1, 3).reshape(B, M, D)
    return np.ascontiguousarray(out).astype(np.float32)
